# revision 8
# baseline (speedup 1.0000x reference)
"""Trainium2 Bass kernel for nn_BaseRecommender (masked top-k recommendation).

Strategy (hardcoded, self-contained):
  - Shard the item table column-wise across 8 cores: 12500 items/core,
    zero-padded to 12800 = 25 matmul chunks x 512.  Replicate
    u_e = all_embed[user_list] (gathered + transposed on host).
  - Matmuls run in bf16 with the contraction dim zero-padded 64 -> 128
    ([u; 0] x [x; junk]): on TRN2 a K=128 bf16 matmul streams ~2.7x
    faster per output column than K=64 (measured 150ns vs 400ns per
    [128,512] output), so the PE pass costs ~3.8us per 128-row tile.
  - The score matrix is NOT reduced on device.  Per 128-row tile the
    7 PSUM chunks are copied f32->fp8e4m3 to SBUF, split between the
    scalar (ACT) and vector (DVE) engines -- the two engines that can
    read PSUM -- at ~1.05 ns/col each, then DMA'd per-chunk to DRAM as
    a full [1024, 12800] fp8 score matrix per core on the SP HWDGE
    queue (fp8 halves the DMA mass so one queue keeps up; the
    ACT-launched queue serializes against the ACT copies).
  - Host: threshold the fp8 scores at (per-row fp8 20th) - 2*MARGIN,
    exactly rescore the ~100-400 surviving candidates per row in f32,
    compute the maskable region (global item cols [0, 1024), the only
    range the reference ever masks) exactly with the min-scatter mask,
    merge, and take a stable tie-aware top-k (lower index first,
    matching jax.lax.top_k).  MARGIN bounds |fp8 device score - exact
    f32 score| (bf16 input rounding + fp8 output quantization, <= ~2.5
    for |s| < 64); the threshold argument makes the result exact, not
    approximate.
"""

import os
import sys

import numpy as np

try:
    import concourse  # noqa: F401
except ImportError:
    for _p in ("/opt/trn_rl_repo", os.path.expanduser("~/.axon_site/_ro/trn_rl_repo")):
        if os.path.isdir(_p):
            sys.path.insert(0, _p)
            try:
                import concourse  # noqa: F401

                break
            except ImportError:
                sys.path.remove(_p)

import ml_dtypes

N_USERS = 100000
N_ITEMS = 100000
EMB = 64
BATCH = 1024
K = 20
NEG = -100000.0
NCORES = 8
ISHARD = N_ITEMS // NCORES  # 12500 items per core
IPAD = 12800  # 25 matmul chunks x 512
ROWT = 128
NROWT = BATCH // ROWT  # 8 row tiles
HOST_COLS = 1024  # item columns [0, HOST_COLS) are scored on host (mask range)
PAIRW = 2048  # psum tile width (4 banks)
NPAIR = 6  # full pair tiles per row tile; 7th tile holds the 512 tail
TAILW = 512
# drain engine per chunk p (0..6): True -> ACT (scalar) copy, False -> DVE copy.
ACT_CHUNKS = (True, False, True, False, True, False, False)
MARGIN = 3.0  # bound on |fp8 device score - exact f32 score| (worst ~2.5)

BF16 = ml_dtypes.bfloat16
F8 = ml_dtypes.float8_e4m3

_compiled = None


def _build_bass(loop_n=1):
    """Per-core Bass program. loop_n > 1 repeats the compute loop (hardware
    For_i) for differential HW timing; input loads happen once."""
    from concourse import bacc
    import concourse.mybir as mybir
    from concourse.tile import TileContext

    F32 = mybir.dt.float32
    MBF16 = mybir.dt.bfloat16
    MF8 = mybir.dt.float8e4

    nc = bacc.Bacc("TRN2", target_bir_lowering=False, debug=False, num_devices=NCORES)
    u_t = nc.dram_tensor("u_t", [128, BATCH], MBF16, kind="ExternalInput")
    i_t = nc.dram_tensor("i_t", [128, IPAD], MBF16, kind="ExternalInput")
    sc = nc.dram_tensor("sc", [BATCH, IPAD], MF8, kind="ExternalOutput")

    with TileContext(nc) as tc:
        with (
            tc.tile_pool(name="consts", bufs=1) as consts,
            tc.tile_pool(name="psum", bufs=1, space="PSUM") as psum,
            tc.tile_pool(name="outb", bufs=2) as outb,
        ):
            u_sb = consts.tile([128, BATCH], MBF16, tag="u_sb")
            nc.sync.dma_start(u_sb[:], u_t[:])
            i_sb = consts.tile([128, IPAD], MBF16, tag="i_sb")
            for c in range(0, IPAD, PAIRW):
                w = min(PAIRW, IPAD - c)
                nc.sync.dma_start(i_sb[:, c : c + w], i_t[:, c : c + w])

            def body():
                for rt in range(NROWT):
                    lhs = u_sb[:, rt * ROWT : (rt + 1) * ROWT]
                    out_sb = outb.tile([ROWT, IPAD], MF8, tag="out_sb")
                    rows = slice(rt * ROWT, (rt + 1) * ROWT)
                    for p in range(NPAIR + 1):
                        base = p * PAIRW
                        w = PAIRW if p < NPAIR else TAILW
                        ps = psum.tile([ROWT, PAIRW], F32, tag=f"ps{p % 2}")
                        for h in range(0, w, 512):
                            nc.tensor.matmul(
                                ps[:, h : h + 512],
                                lhs,
                                i_sb[:, base + h : base + h + 512],
                                start=True,
                                stop=True,
                            )
                        if ACT_CHUNKS[p]:
                            nc.scalar.copy(out_sb[:, base : base + w], ps[:, 0:w])
                        else:
                            nc.vector.tensor_copy(
                                out=out_sb[:, base : base + w], in_=ps[:, 0:w]
                            )
                        nc.sync.dma_start(
                            sc[rows, base : base + w], out_sb[:, base : base + w]
                        )

            if loop_n == 1:
                body()
            else:
                with tc.For_i(0, loop_n, 1):
                    body()

    nc.compile()
    return nc


def _get_compiled():
    global _compiled
    if _compiled is None:
        _compiled = _build_bass()
    return _compiled


def run_device(u_t, i_t_shards, trace=False, **kwargs):
    from concourse.bass_utils import run_bass_kernel_spmd

    nc = _get_compiled()
    in_maps = [{"u_t": u_t, "i_t": i_t_shards[s]} for s in range(NCORES)]
    return run_bass_kernel_spmd(nc, in_maps, list(range(NCORES)), trace=trace, **kwargs)


def make_device_inputs(all_embed, user_list):
    all_embed = np.asarray(all_embed, dtype=np.float32)
    user_list = np.asarray(user_list)
    u_e = all_embed[user_list.astype(np.int64)]  # [BATCH, EMB]
    i_e = all_embed[N_USERS:]  # [I, EMB]
    u_t = np.zeros((128, BATCH), dtype=BF16)
    u_t[:EMB] = u_e.T.astype(BF16)
    i_t_shards = []
    for s in range(NCORES):
        sh = np.zeros((128, IPAD), dtype=BF16)
        sh[:EMB, :ISHARD] = i_e[s * ISHARD : (s + 1) * ISHARD].T.astype(BF16)
        i_t_shards.append(sh)
    return u_e, i_e, u_t, i_t_shards


def _mask_host_scores(s0, pos_pad):
    """Reference masking semantics on the host-scored region: only valid
    positives with local item index < BATCH (== HOST_COLS) are masked."""
    pos_pad = np.asarray(pos_pad)
    item_idx = pos_pad.astype(np.int64) - N_USERS
    valid = (pos_pad >= 0) & (item_idx < HOST_COLS)
    r, c = np.nonzero(valid)
    np.minimum.at(s0, (r, item_idx[r, c]), np.float32(NEG))
    return s0


def postprocess(results, u_e, i_e, pos_pad):
    """Threshold the device bf16 scores, exactly rescore survivors, merge
    with the host-masked region, select the exact global top-K."""
    # Device score matrix [BATCH, N_ITEMS] as f32 (bf16 precision).
    S = np.empty((BATCH, N_ITEMS), dtype=np.float32)
    for s in range(NCORES):
        S[:, s * ISHARD : (s + 1) * ISHARD] = np.asarray(
            results[s]["sc"]
        )[:, :ISHARD].astype(np.float32)
    # The maskable region is handled exactly on host; exclude from device cands.
    S[:, :HOST_COLS] = -np.inf

    # Per-row 20th-largest device score -> threshold that provably captures
    # every item whose exact score could reach the global top-K.
    v20 = -np.partition(-S, K - 1, axis=1)[:, K - 1]
    tau = v20 - 2.0 * MARGIN

    rows, cols = np.nonzero(S >= tau[:, None])
    # Exact f32 rescore of the candidates.
    vals = np.einsum("ce,ce->c", u_e[rows], i_e[cols], optimize=True).astype(np.float32)

    # Pack per-row candidate lists into a padded matrix, columns in ascending
    # global index (rows/cols from nonzero are already row-major sorted).
    counts = np.bincount(rows, minlength=BATCH)
    maxc = int(counts.max())
    offs = np.zeros(BATCH + 1, dtype=np.int64)
    np.cumsum(counts, out=offs[1:])
    pos = np.arange(len(rows)) - offs[rows]
    cand_v = np.full((BATCH, maxc), -np.inf, dtype=np.float32)
    cand_g = np.full((BATCH, maxc), np.int64(1) << 40, dtype=np.int64)
    cand_v[rows, pos] = vals
    cand_g[rows, pos] = cols

    # Host-exact scores for the maskable region (global item cols [0, 1024)).
    s0 = (u_e @ i_e[:HOST_COLS].T).astype(np.float32)
    s0 = _mask_host_scores(s0, pos_pad)

    all_v = np.concatenate([s0, cand_v], axis=1)
    all_g = np.concatenate(
        [
            np.broadcast_to(
                np.arange(HOST_COLS, dtype=np.int64), (BATCH, HOST_COLS)
            ),
            cand_g,
        ],
        axis=1,
    )
    # Columns are already in ascending global index (s0 block, then sorted
    # candidates), so a stable sort on -value reproduces the reference's tie
    # order (lower index first).
    rws = np.arange(BATCH)[:, None]
    order = np.argsort(-all_v, axis=1, kind="stable")[:, :K]
    out_val = all_v[rws, order]
    out_idx = all_g[rws, order]
    return out_idx.astype(np.int32) + N_USERS, out_val


def kernel(all_embed, pos_pad, user_list, k):
    pos_pad = np.asarray(pos_pad)
    k = int(k)
    assert k == K, f"kernel hardcoded for k={K}, got {k}"
    u_e, i_e, u_t, i_t_shards = make_device_inputs(all_embed, user_list)
    res = run_device(u_t, i_t_shards)
    return postprocess(res.results, u_e, i_e, pos_pad)


# revision 10
# speedup vs baseline: 2.7179x; 2.7179x over previous
"""Trainium2 Bass kernel for nn_BaseRecommender (masked top-k recommendation).

Strategy (hardcoded, self-contained):
  - Shard the item table column-wise across 8 cores: 12500 items/core,
    zero-padded to 12800 = 25 matmul chunks x 512.  Replicate
    u_e = all_embed[user_list] (gathered + transposed on host).
  - Matmuls run in bf16 with the contraction dim zero-padded 64 -> 128
    ([u; 0] x [x; junk]): on TRN2 a K=128 bf16 matmul streams ~2.7x
    faster per output column than K=64 (measured 150ns vs 400ns per
    [128,512] output), so the PE pass costs ~3.8us per 128-row tile.
  - The score matrix is NOT reduced on device.  Per 128-row tile the
    7 PSUM chunks ([128,2048] tiles, 4 banks each, 2 rotating) are
    quantized f32 -> fp8e4m3 into SBUF, split between the scalar (ACT)
    engine (copy, ~0.9 ns/col) and the vector (DVE) engine
    (tensor_tensor max against a -inf constant -- the DVE's fast PSUM
    read path at ~1.13 ns/col; a plain DVE tensor_copy from PSUM is
    ~3x slower) -- these are the only two engines that can read PSUM.
  - Each chunk is DMA'd to DRAM right after its drain on the SP HWDGE
    queue into a BLOCK-CONTIGUOUS layout sc[rowtile, chunk, 128, 2048]
    (a row-major [1024, 12800] destination would scatter 128 separate
    4KB rows per DMA and measured ~100x slower).  fp8 keeps the queue
    at ~650ns per chunk so one queue keeps up.
  - Host: reassemble [1024, 100000] fp8-precision scores, threshold at
    (per-row 20th) - 2*MARGIN, exactly rescore the ~100-400 surviving
    candidates per row in f32, compute the maskable region (global
    item cols [0, 1024), the only range the reference ever masks)
    exactly with the min-scatter mask, merge, and take a stable
    tie-aware top-k (lower index first, matching jax.lax.top_k).
    MARGIN bounds |fp8 device score - exact f32 score| (bf16 input
    rounding + fp8 output quantization, <= ~2.5 for |s| < 64); the
    threshold argument makes the result exact, not approximate.
"""

import os
import sys

import numpy as np

try:
    import concourse  # noqa: F401
except ImportError:
    for _p in ("/opt/trn_rl_repo", os.path.expanduser("~/.axon_site/_ro/trn_rl_repo")):
        if os.path.isdir(_p):
            sys.path.insert(0, _p)
            try:
                import concourse  # noqa: F401

                break
            except ImportError:
                sys.path.remove(_p)

import ml_dtypes

N_USERS = 100000
N_ITEMS = 100000
EMB = 64
BATCH = 1024
K = 20
NEG = -100000.0
NCORES = 8
ISHARD = N_ITEMS // NCORES  # 12500 items per core
IPAD = 12800  # 25 matmul chunks x 512
ROWT = 128
NROWT = BATCH // ROWT  # 8 row tiles
HOST_COLS = 1024  # item columns [0, HOST_COLS) are scored on host (mask range)
PAIRW = 2048  # psum tile width (4 banks)
NCH = 7  # psum chunks per row tile; the 7th holds only the 512-col tail
TAILW = 512
# drain engine per chunk p (0..6): True -> ACT (scalar) copy, False -> DVE
# tensor_tensor(max, -inf const, psum).
ACT_CHUNKS = (True, False, True, False, True, False, True)
MARGIN = 3.0  # bound on |fp8 device score - exact f32 score| (worst ~2.5)

BF16 = ml_dtypes.bfloat16

_compiled = None


def _build_bass(loop_n=1):
    """Per-core Bass program. loop_n > 1 repeats the compute loop (hardware
    For_i) for differential HW timing; input loads happen once."""
    from concourse import bacc
    import concourse.mybir as mybir
    from concourse.tile import TileContext

    F32 = mybir.dt.float32
    MBF16 = mybir.dt.bfloat16
    MF8 = mybir.dt.float8e4
    MAX = mybir.AluOpType.max

    nc = bacc.Bacc("TRN2", target_bir_lowering=False, debug=False, num_devices=NCORES)
    u_t = nc.dram_tensor("u_t", [128, BATCH], MBF16, kind="ExternalInput")
    i_t = nc.dram_tensor("i_t", [128, IPAD], MBF16, kind="ExternalInput")
    sc = nc.dram_tensor("sc", [NROWT, NCH, ROWT, PAIRW], MF8, kind="ExternalOutput")

    with TileContext(nc) as tc:
        with (
            tc.tile_pool(name="consts", bufs=1) as consts,
            tc.tile_pool(name="psum", bufs=1, space="PSUM") as psum,
            tc.tile_pool(name="outb", bufs=2) as outb,
        ):
            u_sb = consts.tile([128, BATCH], MBF16, tag="u_sb")
            nc.sync.dma_start(u_sb[:], u_t[:])
            i_sb = consts.tile([128, IPAD], MBF16, tag="i_sb")
            for c in range(0, IPAD, PAIRW):
                w = min(PAIRW, IPAD - c)
                nc.sync.dma_start(i_sb[:, c : c + w], i_t[:, c : c + w])
            neg8 = consts.tile([ROWT, PAIRW], MF8, tag="neg8")
            nc.vector.memset(neg8[:], -240.0)

            def body():
                for rt in range(NROWT):
                    lhs = u_sb[:, rt * ROWT : (rt + 1) * ROWT]
                    out_sb = outb.tile([ROWT, NCH * PAIRW], MF8, tag="out_sb")
                    for p in range(NCH):
                        base = p * PAIRW
                        w = PAIRW if p < NCH - 1 else TAILW
                        ps = psum.tile([ROWT, PAIRW], F32, tag=f"ps{p % 2}")
                        for h in range(0, w, 512):
                            nc.tensor.matmul(
                                ps[:, h : h + 512],
                                lhs,
                                i_sb[:, base + h : base + h + 512],
                                start=True,
                                stop=True,
                            )
                        dst = out_sb[:, base : base + w]
                        if ACT_CHUNKS[p]:
                            nc.scalar.copy(dst, ps[:, 0:w])
                        else:
                            nc.vector.tensor_tensor(
                                dst, neg8[:, 0:w], ps[:, 0:w], op=MAX
                            )
                        nc.sync.dma_start(
                            sc[rt, p, :, 0:w], out_sb[:, base : base + w]
                        )

            if loop_n == 1:
                body()
            else:
                with tc.For_i(0, loop_n, 1):
                    body()

    nc.compile()
    return nc


def _get_compiled():
    global _compiled
    if _compiled is None:
        _compiled = _build_bass()
    return _compiled


def run_device(u_t, i_t_shards, trace=False, **kwargs):
    from concourse.bass_utils import run_bass_kernel_spmd

    nc = _get_compiled()
    in_maps = [{"u_t": u_t, "i_t": i_t_shards[s]} for s in range(NCORES)]
    return run_bass_kernel_spmd(nc, in_maps, list(range(NCORES)), trace=trace, **kwargs)


def make_device_inputs(all_embed, user_list):
    all_embed = np.asarray(all_embed, dtype=np.float32)
    user_list = np.asarray(user_list)
    u_e = all_embed[user_list.astype(np.int64)]  # [BATCH, EMB]
    i_e = all_embed[N_USERS:]  # [I, EMB]
    u_t = np.zeros((128, BATCH), dtype=BF16)
    u_t[:EMB] = u_e.T.astype(BF16)
    i_t_shards = []
    for s in range(NCORES):
        sh = np.zeros((128, IPAD), dtype=BF16)
        sh[:EMB, :ISHARD] = i_e[s * ISHARD : (s + 1) * ISHARD].T.astype(BF16)
        i_t_shards.append(sh)
    return u_e, i_e, u_t, i_t_shards


def _mask_host_scores(s0, pos_pad):
    """Reference masking semantics on the host-scored region: only valid
    positives with local item index < BATCH (== HOST_COLS) are masked."""
    pos_pad = np.asarray(pos_pad)
    item_idx = pos_pad.astype(np.int64) - N_USERS
    valid = (pos_pad >= 0) & (item_idx < HOST_COLS)
    r, c = np.nonzero(valid)
    np.minimum.at(s0, (r, item_idx[r, c]), np.float32(NEG))
    return s0


def _assemble_scores(results):
    """Device fp8 scores -> [BATCH, N_ITEMS] f32."""
    S = np.empty((BATCH, N_ITEMS), dtype=np.float32)
    for s in range(NCORES):
        arr = np.asarray(results[s]["sc"])  # [NROWT, NCH, ROWT, PAIRW]
        parts = [arr[:, p, :, :] for p in range(NCH - 1)] + [arr[:, NCH - 1, :, :TAILW]]
        core = np.concatenate(parts, axis=2).reshape(BATCH, IPAD)
        S[:, s * ISHARD : (s + 1) * ISHARD] = core[:, :ISHARD].astype(np.float32)
    return S


def postprocess(results, u_e, i_e, pos_pad):
    """Threshold the device scores, exactly rescore survivors, merge with
    the host-masked region, select the exact global top-K."""
    S = _assemble_scores(results)
    # The maskable region is handled exactly on host; exclude from device cands.
    S[:, :HOST_COLS] = -np.inf

    # Per-row 20th-largest device score -> threshold that provably captures
    # every item whose exact score could reach the global top-K.
    v20 = -np.partition(-S, K - 1, axis=1)[:, K - 1]
    tau = v20 - 2.0 * MARGIN

    rows, cols = np.nonzero(S >= tau[:, None])
    # Exact f32 rescore of the candidates.
    vals = np.einsum("ce,ce->c", u_e[rows], i_e[cols], optimize=True).astype(np.float32)

    # Pack per-row candidate lists into a padded matrix, columns in ascending
    # global index (rows/cols from nonzero are already row-major sorted).
    counts = np.bincount(rows, minlength=BATCH)
    maxc = int(counts.max())
    offs = np.zeros(BATCH + 1, dtype=np.int64)
    np.cumsum(counts, out=offs[1:])
    pos = np.arange(len(rows)) - offs[rows]
    cand_v = np.full((BATCH, maxc), -np.inf, dtype=np.float32)
    cand_g = np.full((BATCH, maxc), np.int64(1) << 40, dtype=np.int64)
    cand_v[rows, pos] = vals
    cand_g[rows, pos] = cols

    # Host-exact scores for the maskable region (global item cols [0, 1024)).
    s0 = (u_e @ i_e[:HOST_COLS].T).astype(np.float32)
    s0 = _mask_host_scores(s0, pos_pad)

    all_v = np.concatenate([s0, cand_v], axis=1)
    all_g = np.concatenate(
        [
            np.broadcast_to(
                np.arange(HOST_COLS, dtype=np.int64), (BATCH, HOST_COLS)
            ),
            cand_g,
        ],
        axis=1,
    )
    # Columns are already in ascending global index (s0 block, then sorted
    # candidates), so a stable sort on -value reproduces the reference's tie
    # order (lower index first).
    rws = np.arange(BATCH)[:, None]
    order = np.argsort(-all_v, axis=1, kind="stable")[:, :K]
    out_val = all_v[rws, order]
    out_idx = all_g[rws, order]
    return out_idx.astype(np.int32) + N_USERS, out_val


def kernel(all_embed, pos_pad, user_list, k):
    pos_pad = np.asarray(pos_pad)
    k = int(k)
    assert k == K, f"kernel hardcoded for k={K}, got {k}"
    u_e, i_e, u_t, i_t_shards = make_device_inputs(all_embed, user_list)
    res = run_device(u_t, i_t_shards)
    return postprocess(res.results, u_e, i_e, pos_pad)


# revision 14
# speedup vs baseline: 3.5007x; 1.2880x over previous
"""Trainium2 Bass kernel for nn_BaseRecommender (masked top-k recommendation).

Strategy (hardcoded, self-contained):
  - Shard the item table column-wise across 8 cores: 12500 items/core,
    zero-padded to 12800 = 25 matmul chunks x 512.  Replicate
    u_e = all_embed[user_list] (gathered + transposed on host).
  - Matmuls run in bf16 with the contraction dim zero-padded 64 -> 128
    ([u; 0] x [x; junk]): on TRN2 a K=128 bf16 matmul streams ~2.7x
    faster per output column than K=64 (measured 150ns vs 400ns per
    [128,512] output), so the PE pass costs ~3.8us per 128-row tile.
  - The score matrix is NOT reduced on device.  Per 128-row tile the
    7 PSUM chunks ([128,2048] tiles, 4 banks each, 2 rotating) are
    quantized f32 -> fp8e4m3 into SBUF, split between the scalar (ACT)
    engine (copy, ~0.9 ns/col) and the vector (DVE) engine
    (tensor_tensor max against a -inf constant -- the DVE's fast PSUM
    read path at ~1.13 ns/col; a plain DVE tensor_copy from PSUM is
    ~3x slower) -- these are the only two engines that can read PSUM.
  - Each chunk is DMA'd to DRAM right after its drain on the SP HWDGE
    queue into a BLOCK-CONTIGUOUS layout sc[rowtile, chunk, 128, 2048]
    (a row-major [1024, 12800] destination would scatter 128 separate
    4KB rows per DMA and measured ~100x slower).  fp8 keeps the queue
    at ~650ns per chunk so one queue keeps up.
  - Host: reassemble [1024, 100000] fp8-precision scores, threshold at
    (per-row 20th) - 2*MARGIN, exactly rescore the ~100-400 surviving
    candidates per row in f32, compute the maskable region (global
    item cols [0, 1024), the only range the reference ever masks)
    exactly with the min-scatter mask, merge, and take a stable
    tie-aware top-k (lower index first, matching jax.lax.top_k).
    MARGIN bounds |fp8 device score - exact f32 score| (bf16 input
    rounding + fp8 output quantization, <= ~2.5 for |s| < 64); the
    threshold argument makes the result exact, not approximate.
"""

import os
import sys

import numpy as np

try:
    import concourse  # noqa: F401
except ImportError:
    for _p in ("/opt/trn_rl_repo", os.path.expanduser("~/.axon_site/_ro/trn_rl_repo")):
        if os.path.isdir(_p):
            sys.path.insert(0, _p)
            try:
                import concourse  # noqa: F401

                break
            except ImportError:
                sys.path.remove(_p)

import ml_dtypes

N_USERS = 100000
N_ITEMS = 100000
EMB = 64
BATCH = 1024
K = 20
NEG = -100000.0
NCORES = 8
ISHARD = N_ITEMS // NCORES  # 12500 items per core
IPAD = 12800  # 25 matmul chunks x 512
ROWT = 128
NROWT = BATCH // ROWT  # 8 row tiles
HOST_COLS = 1024  # item columns [0, HOST_COLS) are scored on host (mask range)
PAIRW = 2048  # DMA block width (2 drain chunks)
NBLK = 7  # DMA blocks per row tile; the 7th holds only the 512-col tail
CW = 1024  # psum tile / drain chunk width (2 banks; 4 tiles rotate)
NCH = 13  # drain chunks per row tile (12 full + 512-col tail)
NTILE = 4  # psum tiles in flight
TAILW = 512
MARGIN = 3.0  # bound on |fp8 device score - exact f32 score| (worst ~2.5)

BF16 = ml_dtypes.bfloat16

_compiled = None


def _build_bass(loop_n=1):
    """Per-core Bass program. loop_n > 1 repeats the compute loop (hardware
    For_i) for differential HW timing; input loads happen once."""
    from concourse import bacc
    import concourse.mybir as mybir
    from concourse.tile import TileContext

    F32 = mybir.dt.float32
    MBF16 = mybir.dt.bfloat16
    MF8 = mybir.dt.float8e4
    MAX = mybir.AluOpType.max

    nc = bacc.Bacc("TRN2", target_bir_lowering=False, debug=False, num_devices=NCORES)
    u_t = nc.dram_tensor("u_t", [128, BATCH], MBF16, kind="ExternalInput")
    i_t = nc.dram_tensor("i_t", [128, IPAD], MBF16, kind="ExternalInput")
    sc = nc.dram_tensor("sc", [NROWT, NBLK, ROWT, PAIRW], MF8, kind="ExternalOutput")

    with TileContext(nc) as tc:
        with (
            tc.tile_pool(name="consts", bufs=1) as consts,
            tc.tile_pool(name="psum", bufs=1, space="PSUM") as psum,
            tc.tile_pool(name="outb", bufs=2) as outb,
        ):
            u_sb = consts.tile([128, BATCH], MBF16, tag="u_sb")
            nc.sync.dma_start(u_sb[:], u_t[:])
            i_sb = consts.tile([128, IPAD], MBF16, tag="i_sb")
            for c in range(0, IPAD, PAIRW):
                w = min(PAIRW, IPAD - c)
                nc.sync.dma_start(i_sb[:, c : c + w], i_t[:, c : c + w])
            neg8 = consts.tile([ROWT, CW], MF8, tag="neg8")
            nc.vector.memset(neg8[:], -240.0)

            def body():
                for rt in range(NROWT):
                    lhs = u_sb[:, rt * ROWT : (rt + 1) * ROWT]
                    out_sb = outb.tile([ROWT, NBLK * PAIRW], MF8, tag="out_sb")
                    for p in range(NCH):
                        base = p * CW
                        w = CW if p < NCH - 1 else TAILW
                        ps = psum.tile([ROWT, CW], F32, tag=f"ps{p % NTILE}")
                        for h in range(0, w, 512):
                            nc.tensor.matmul(
                                ps[:, h : h + 512],
                                lhs,
                                i_sb[:, base + h : base + h + 512],
                                start=True,
                                stop=True,
                            )
                        dst = out_sb[:, base : base + w]
                        if p % 2 == 0:
                            nc.scalar.copy(dst, ps[:, 0:w])
                        else:
                            nc.vector.tensor_tensor(
                                dst, neg8[:, 0:w], ps[:, 0:w], op=MAX
                            )
                        # one DMA per 2048-wide block (2 chunks), right after
                        # the block's second drain; the tail block is 512 wide
                        if p % 2 == 1 or p == NCH - 1:
                            blk = p // 2
                            bw = PAIRW if p % 2 == 1 else TAILW
                            nc.sync.dma_start(
                                sc[rt, blk, :, 0:bw],
                                out_sb[:, blk * PAIRW : blk * PAIRW + bw],
                            )

            if loop_n == 1:
                body()
            else:
                with tc.For_i(0, loop_n, 1):
                    body()

    nc.compile()
    return nc


def _get_compiled():
    global _compiled
    if _compiled is None:
        _compiled = _build_bass()
    return _compiled


def run_device(u_t, i_t_shards, trace=False, **kwargs):
    from concourse.bass_utils import run_bass_kernel_spmd

    nc = _get_compiled()
    in_maps = [{"u_t": u_t, "i_t": i_t_shards[s]} for s in range(NCORES)]
    return run_bass_kernel_spmd(nc, in_maps, list(range(NCORES)), trace=trace, **kwargs)


def make_device_inputs(all_embed, user_list):
    all_embed = np.asarray(all_embed, dtype=np.float32)
    user_list = np.asarray(user_list)
    u_e = all_embed[user_list.astype(np.int64)]  # [BATCH, EMB]
    i_e = all_embed[N_USERS:]  # [I, EMB]
    u_t = np.zeros((128, BATCH), dtype=BF16)
    u_t[:EMB] = u_e.T.astype(BF16)
    i_t_shards = []
    for s in range(NCORES):
        sh = np.zeros((128, IPAD), dtype=BF16)
        sh[:EMB, :ISHARD] = i_e[s * ISHARD : (s + 1) * ISHARD].T.astype(BF16)
        i_t_shards.append(sh)
    return u_e, i_e, u_t, i_t_shards


def _mask_host_scores(s0, pos_pad):
    """Reference masking semantics on the host-scored region: only valid
    positives with local item index < BATCH (== HOST_COLS) are masked."""
    pos_pad = np.asarray(pos_pad)
    item_idx = pos_pad.astype(np.int64) - N_USERS
    valid = (pos_pad >= 0) & (item_idx < HOST_COLS)
    r, c = np.nonzero(valid)
    np.minimum.at(s0, (r, item_idx[r, c]), np.float32(NEG))
    return s0


def _assemble_scores(results):
    """Device fp8 scores -> [BATCH, N_ITEMS] f32."""
    S = np.empty((BATCH, N_ITEMS), dtype=np.float32)
    for s in range(NCORES):
        arr = np.asarray(results[s]["sc"])  # [NROWT, NBLK, ROWT, PAIRW]
        parts = [arr[:, b, :, :] for b in range(NBLK - 1)] + [
            arr[:, NBLK - 1, :, :TAILW]
        ]
        core = np.concatenate(parts, axis=2).reshape(BATCH, IPAD)
        S[:, s * ISHARD : (s + 1) * ISHARD] = core[:, :ISHARD].astype(np.float32)
    return S


def postprocess(results, u_e, i_e, pos_pad):
    """Threshold the device scores, exactly rescore survivors, merge with
    the host-masked region, select the exact global top-K."""
    S = _assemble_scores(results)
    # The maskable region is handled exactly on host; exclude from device cands.
    S[:, :HOST_COLS] = -np.inf

    # Per-row 20th-largest device score -> threshold that provably captures
    # every item whose exact score could reach the global top-K.
    v20 = -np.partition(-S, K - 1, axis=1)[:, K - 1]
    tau = v20 - 2.0 * MARGIN

    rows, cols = np.nonzero(S >= tau[:, None])
    # Exact f32 rescore of the candidates.
    vals = np.einsum("ce,ce->c", u_e[rows], i_e[cols], optimize=True).astype(np.float32)

    # Pack per-row candidate lists into a padded matrix, columns in ascending
    # global index (rows/cols from nonzero are already row-major sorted).
    counts = np.bincount(rows, minlength=BATCH)
    maxc = int(counts.max())
    offs = np.zeros(BATCH + 1, dtype=np.int64)
    np.cumsum(counts, out=offs[1:])
    pos = np.arange(len(rows)) - offs[rows]
    cand_v = np.full((BATCH, maxc), -np.inf, dtype=np.float32)
    cand_g = np.full((BATCH, maxc), np.int64(1) << 40, dtype=np.int64)
    cand_v[rows, pos] = vals
    cand_g[rows, pos] = cols

    # Host-exact scores for the maskable region (global item cols [0, 1024)).
    s0 = (u_e @ i_e[:HOST_COLS].T).astype(np.float32)
    s0 = _mask_host_scores(s0, pos_pad)

    all_v = np.concatenate([s0, cand_v], axis=1)
    all_g = np.concatenate(
        [
            np.broadcast_to(
                np.arange(HOST_COLS, dtype=np.int64), (BATCH, HOST_COLS)
            ),
            cand_g,
        ],
        axis=1,
    )
    # Columns are already in ascending global index (s0 block, then sorted
    # candidates), so a stable sort on -value reproduces the reference's tie
    # order (lower index first).
    rws = np.arange(BATCH)[:, None]
    order = np.argsort(-all_v, axis=1, kind="stable")[:, :K]
    out_val = all_v[rws, order]
    out_idx = all_g[rws, order]
    return out_idx.astype(np.int32) + N_USERS, out_val


def kernel(all_embed, pos_pad, user_list, k):
    pos_pad = np.asarray(pos_pad)
    k = int(k)
    assert k == K, f"kernel hardcoded for k={K}, got {k}"
    u_e, i_e, u_t, i_t_shards = make_device_inputs(all_embed, user_list)
    res = run_device(u_t, i_t_shards)
    return postprocess(res.results, u_e, i_e, pos_pad)


# revision 17
# speedup vs baseline: 5.0434x; 1.4407x over previous
"""Trainium2 Bass kernel for nn_BaseRecommender (masked top-k recommendation).

Strategy (hardcoded, self-contained):
  - Shard the item table column-wise across 8 cores: 12500 items/core,
    zero-padded to 12800 = 25 matmul chunks x 512.  Replicate
    u_e = all_embed[user_list] (gathered + transposed on host).
  - Matmuls run in bf16 with the contraction dim zero-padded 64 -> 128
    ([u; 0] x [x; junk]): on TRN2 a K=128 bf16 matmul streams ~2.7x
    faster per output column than K=64 (measured 150ns vs 400ns per
    [128,512] output), so the PE pass costs ~3.8us per 128-row tile.
  - The score matrix is NOT reduced on device.  Per 128-row tile the
    7 PSUM chunks ([128,2048] tiles, 4 banks each, 2 rotating) are
    quantized f32 -> fp8e4m3 into SBUF, split between the scalar (ACT)
    engine (copy, ~0.9 ns/col) and the vector (DVE) engine
    (tensor_tensor max against a -inf constant -- the DVE's fast PSUM
    read path at ~1.13 ns/col; a plain DVE tensor_copy from PSUM is
    ~3x slower) -- these are the only two engines that can read PSUM.
  - Each chunk is DMA'd to DRAM right after its drain on the SP HWDGE
    queue into a BLOCK-CONTIGUOUS layout sc[rowtile, chunk, 128, 2048]
    (a row-major [1024, 12800] destination would scatter 128 separate
    4KB rows per DMA and measured ~100x slower).  fp8 keeps the queue
    at ~650ns per chunk so one queue keeps up.
  - Host: reassemble [1024, 100000] fp8-precision scores, threshold at
    (per-row 20th) - 2*MARGIN, exactly rescore the ~100-400 surviving
    candidates per row in f32, compute the maskable region (global
    item cols [0, 1024), the only range the reference ever masks)
    exactly with the min-scatter mask, merge, and take a stable
    tie-aware top-k (lower index first, matching jax.lax.top_k).
    MARGIN bounds |fp8 device score - exact f32 score| (bf16 input
    rounding + fp8 output quantization, <= ~2.5 for |s| < 64); the
    threshold argument makes the result exact, not approximate.
"""

import os
import sys

import numpy as np

try:
    import concourse  # noqa: F401
except ImportError:
    for _p in ("/opt/trn_rl_repo", os.path.expanduser("~/.axon_site/_ro/trn_rl_repo")):
        if os.path.isdir(_p):
            sys.path.insert(0, _p)
            try:
                import concourse  # noqa: F401

                break
            except ImportError:
                sys.path.remove(_p)

import ml_dtypes

N_USERS = 100000
N_ITEMS = 100000
EMB = 64
BATCH = 1024
K = 20
NEG = -100000.0
NCORES = 8
ISHARD = N_ITEMS // NCORES  # 12500 items per core
IPAD = 12544  # 12 chunks x 1024 + 256-col tail
ROWT = 128
NROWT = BATCH // ROWT  # 8 row tiles
HOST_COLS = 1024  # item columns [0, HOST_COLS) are scored on host (mask range)
PAIRW = 2048  # DMA block width (2 drain chunks)
NBLK = 7  # DMA blocks per row tile; the 7th holds only the 256-col tail
CW = 1024  # psum tile / drain chunk width (2 banks; 4 tiles rotate)
NCH = 13  # drain chunks per row tile (12 full + 256-col tail)
NTILE = 4  # psum tiles in flight
TAILW = 256
# chunks drained by the ACT (scalar) engine; the rest (incl. the cheap tail)
# go to the DVE.  ACT is slightly faster per column on HW, so it takes 7.
ACT_SET = frozenset((0, 2, 4, 6, 8, 10, 11))
MARGIN = 3.0  # bound on |fp8 device score - exact f32 score| (worst ~2.5)

BF16 = ml_dtypes.bfloat16

_compiled = None


def _build_bass(loop_n=1):
    """Per-core Bass program. loop_n > 1 repeats the compute loop (hardware
    For_i) for differential HW timing; input loads happen once."""
    from concourse import bacc
    import concourse.mybir as mybir
    from concourse.tile import TileContext

    F32 = mybir.dt.float32
    MBF16 = mybir.dt.bfloat16
    MF8 = mybir.dt.float8e4
    MAX = mybir.AluOpType.max

    nc = bacc.Bacc("TRN2", target_bir_lowering=False, debug=False, num_devices=NCORES)
    u_t = nc.dram_tensor("u_t", [128, BATCH], MBF16, kind="ExternalInput")
    i_t = nc.dram_tensor("i_t", [128, IPAD], MBF16, kind="ExternalInput")
    sc = nc.dram_tensor("sc", [NROWT, NBLK, ROWT, PAIRW], MF8, kind="ExternalOutput")

    with TileContext(nc) as tc:
        with (
            tc.tile_pool(name="consts", bufs=1) as consts,
            tc.tile_pool(name="psum", bufs=1, space="PSUM") as psum,
            tc.tile_pool(name="outb", bufs=2) as outb,
        ):
            u_sb = consts.tile([128, BATCH], MBF16, tag="u_sb")
            nc.sync.dma_start(u_sb[:], u_t[:])
            i_sb = consts.tile([128, IPAD], MBF16, tag="i_sb")
            for c in range(0, IPAD, PAIRW):
                w = min(PAIRW, IPAD - c)
                nc.sync.dma_start(i_sb[:, c : c + w], i_t[:, c : c + w])
            neg8 = consts.tile([ROWT, CW], MF8, tag="neg8")
            nc.vector.memset(neg8[:], -240.0)

            def body():
                for rt in range(NROWT):
                    lhs = u_sb[:, rt * ROWT : (rt + 1) * ROWT]
                    out_sb = outb.tile([ROWT, NBLK * PAIRW], MF8, tag="out_sb")
                    for p in range(NCH):
                        base = p * CW
                        w = CW if p < NCH - 1 else TAILW
                        ps = psum.tile([ROWT, CW], F32, tag=f"ps{p % NTILE}")
                        for h in range(0, w, 512):
                            hw_ = min(512, w - h)
                            nc.tensor.matmul(
                                ps[:, h : h + hw_],
                                lhs,
                                i_sb[:, base + h : base + h + hw_],
                                start=True,
                                stop=True,
                            )
                        dst = out_sb[:, base : base + w]
                        if p in ACT_SET:
                            nc.scalar.copy(dst, ps[:, 0:w])
                        else:
                            nc.vector.tensor_tensor(
                                dst, neg8[:, 0:w], ps[:, 0:w], op=MAX
                            )
                        # one DMA per 2048-wide block (2 chunks), right after
                        # the block's second drain; the tail block is 512 wide
                        if p % 2 == 1 or p == NCH - 1:
                            blk = p // 2
                            bw = PAIRW if p % 2 == 1 else TAILW
                            nc.sync.dma_start(
                                sc[rt, blk, :, 0:bw],
                                out_sb[:, blk * PAIRW : blk * PAIRW + bw],
                            )

            if loop_n == 1:
                body()
            else:
                with tc.For_i(0, loop_n, 1):
                    body()

    nc.compile()
    return nc


def _get_compiled():
    global _compiled
    if _compiled is None:
        _compiled = _build_bass()
    return _compiled


def run_device(u_t, i_t_shards, trace=False, **kwargs):
    from concourse.bass_utils import run_bass_kernel_spmd

    nc = _get_compiled()
    in_maps = [{"u_t": u_t, "i_t": i_t_shards[s]} for s in range(NCORES)]
    return run_bass_kernel_spmd(nc, in_maps, list(range(NCORES)), trace=trace, **kwargs)


def make_device_inputs(all_embed, user_list):
    all_embed = np.asarray(all_embed, dtype=np.float32)
    user_list = np.asarray(user_list)
    u_e = all_embed[user_list.astype(np.int64)]  # [BATCH, EMB]
    i_e = all_embed[N_USERS:]  # [I, EMB]
    u_t = np.zeros((128, BATCH), dtype=BF16)
    u_t[:EMB] = u_e.T.astype(BF16)
    i_t_shards = []
    for s in range(NCORES):
        sh = np.zeros((128, IPAD), dtype=BF16)
        sh[:EMB, :ISHARD] = i_e[s * ISHARD : (s + 1) * ISHARD].T.astype(BF16)
        i_t_shards.append(sh)
    return u_e, i_e, u_t, i_t_shards


def _mask_host_scores(s0, pos_pad):
    """Reference masking semantics on the host-scored region: only valid
    positives with local item index < BATCH (== HOST_COLS) are masked."""
    pos_pad = np.asarray(pos_pad)
    item_idx = pos_pad.astype(np.int64) - N_USERS
    valid = (pos_pad >= 0) & (item_idx < HOST_COLS)
    r, c = np.nonzero(valid)
    np.minimum.at(s0, (r, item_idx[r, c]), np.float32(NEG))
    return s0


def _assemble_scores(results):
    """Device fp8 scores -> [BATCH, N_ITEMS] f32."""
    S = np.empty((BATCH, N_ITEMS), dtype=np.float32)
    for s in range(NCORES):
        arr = np.asarray(results[s]["sc"])  # [NROWT, NBLK, ROWT, PAIRW]
        parts = [arr[:, b, :, :] for b in range(NBLK - 1)] + [
            arr[:, NBLK - 1, :, :TAILW]
        ]
        core = np.concatenate(parts, axis=2).reshape(BATCH, IPAD)
        S[:, s * ISHARD : (s + 1) * ISHARD] = core[:, :ISHARD].astype(np.float32)
    return S


def postprocess(results, u_e, i_e, pos_pad):
    """Threshold the device scores, exactly rescore survivors, merge with
    the host-masked region, select the exact global top-K."""
    S = _assemble_scores(results)
    # The maskable region is handled exactly on host; exclude from device cands.
    S[:, :HOST_COLS] = -np.inf

    # Per-row 20th-largest device score -> threshold that provably captures
    # every item whose exact score could reach the global top-K.
    v20 = -np.partition(-S, K - 1, axis=1)[:, K - 1]
    tau = v20 - 2.0 * MARGIN

    rows, cols = np.nonzero(S >= tau[:, None])
    # Exact f32 rescore of the candidates.
    vals = np.einsum("ce,ce->c", u_e[rows], i_e[cols], optimize=True).astype(np.float32)

    # Pack per-row candidate lists into a padded matrix, columns in ascending
    # global index (rows/cols from nonzero are already row-major sorted).
    counts = np.bincount(rows, minlength=BATCH)
    maxc = int(counts.max())
    offs = np.zeros(BATCH + 1, dtype=np.int64)
    np.cumsum(counts, out=offs[1:])
    pos = np.arange(len(rows)) - offs[rows]
    cand_v = np.full((BATCH, maxc), -np.inf, dtype=np.float32)
    cand_g = np.full((BATCH, maxc), np.int64(1) << 40, dtype=np.int64)
    cand_v[rows, pos] = vals
    cand_g[rows, pos] = cols

    # Host-exact scores for the maskable region (global item cols [0, 1024)).
    s0 = (u_e @ i_e[:HOST_COLS].T).astype(np.float32)
    s0 = _mask_host_scores(s0, pos_pad)

    all_v = np.concatenate([s0, cand_v], axis=1)
    all_g = np.concatenate(
        [
            np.broadcast_to(
                np.arange(HOST_COLS, dtype=np.int64), (BATCH, HOST_COLS)
            ),
            cand_g,
        ],
        axis=1,
    )
    # Columns are already in ascending global index (s0 block, then sorted
    # candidates), so a stable sort on -value reproduces the reference's tie
    # order (lower index first).
    rws = np.arange(BATCH)[:, None]
    order = np.argsort(-all_v, axis=1, kind="stable")[:, :K]
    out_val = all_v[rws, order]
    out_idx = all_g[rws, order]
    return out_idx.astype(np.int32) + N_USERS, out_val


def kernel(all_embed, pos_pad, user_list, k):
    pos_pad = np.asarray(pos_pad)
    k = int(k)
    assert k == K, f"kernel hardcoded for k={K}, got {k}"
    u_e, i_e, u_t, i_t_shards = make_device_inputs(all_embed, user_list)
    res = run_device(u_t, i_t_shards)
    return postprocess(res.results, u_e, i_e, pos_pad)
